# revision 45
# baseline (speedup 1.0000x reference)
"""GCN-style 8-step SpMM power iteration on 8 Trainium2 NeuronCores.

Math (reference):
    deg = segment_sum(1, col); dis = rsqrt(max(deg,1)) where deg>0 else 0
    norm_e = dis[row_e] * dis[col_e];  row' = row - row.min()
    xX = x @ W_linX + b_linX
    hX_{t+1}[v] = sum_{e: row'_e = v} norm_e * hX_t[col_e] + xX[v]   (8 times)
    out = relu(pp0*xX + pp1*hX_8) @ W_pred + b_pred

Key algebraic trick: norm factorizes per-edge into src/dst node factors, so we
keep the node table pre-scaled: T = dis ⊙ hX. Then one step is
    S[v]   = sum_{e->v} T[col_e]            (pure gather + segment-sum, no
                                             per-edge arithmetic at all)
    T'[v]  = (dis*dis_sh)[v]*S[v] + (dis*xX)[v]   (one DVE op per group)

Distribution: nodes dst-sharded over 8 cores; edges partitioned by dst core.
Nodes are relabeled into "slots": 32-slot sub-blocks (bins) packed so each
bin's in-edges fit a variable number of 128-edge chunks; 4 bins = one PSUM
group of 128 dst rows. Per iteration each core stages its T' shard (fp8) and
one AllGather rebuilds the full table on every core at the iteration
boundary. The boundary AG runs in a quiet window (the first gather of the
next iteration head-of-line parks the in-order Pool sequencer until the AG
lands) because concurrent SWDGE gather traffic inflates collective latency
~3x (small random reads keep the DMA engines latency-saturated).

The table is fp8 (e4m3); rows are gathered in PAIRS (256B descriptors - the
dma_gather minimum) and a per-chunk [even|odd] pair of 32-col fp8 selection
blocks picks the right row of each gathered pair on the TensorEngine,
accumulating fp32 in PSUM. Descriptor generation on the 4 SWDGE queues
(~2ns/idx aggregate) and the DMA engines' random-read latency are the
co-limiting resources; total gathered indices are minimized by per-bin
variable chunk counts (cross-core aligned by sorted position).
"""

import numpy as np

# problem shape (hardcoded per the task contract)
N = 50000
E = 800000
IN_C = 128
HID = 128
OUT_C = 40
POWER1 = 8

NCORES = 8
SUB_NODES = 32          # destination slots per sub-block (= matmul M)
CHUNK = 128             # edges per chunk (= matmul K)
GROUP_SUBS = 4          # sub-blocks per psum group ([128,128] psum tile)
CAP_EDGES = 512         # max in-edges per bin (4 chunks)
BATCH_CHUNKS = 8        # max chunks per gather call (1024 idx)
NQUEUES = 4             # SWDGE queues used for the gathers


# ----------------------------------------------------------------------------
# Host-side preprocessing
# ----------------------------------------------------------------------------

def _pack_core(deg, cap, sub_nodes):
    """1D best-fit-decreasing packing of nodes into bins (<=sub_nodes nodes,
    <=cap edges)."""
    order = np.argsort(-deg, kind="stable")
    bins = []        # (node_list, sum_edges)
    for v in order:
        a = int(deg[v])
        best, best_slack = -1, None
        for i, (nodes, sa) in enumerate(bins):
            if len(nodes) < sub_nodes and sa + a <= cap:
                slack = cap - sa - a
                if best_slack is None or slack < best_slack:
                    best, best_slack = i, slack
        if best < 0:
            bins.append(([v], a))
        else:
            nodes, sa = bins[best]
            nodes.append(v)
            bins[best] = (nodes, sa + a)
    return bins


def _preprocess(inputs, n=N, ncores=NCORES):
    import ml_dtypes
    x = np.asarray(inputs["x"], dtype=np.float32)
    edge_index = np.asarray(inputs["edge_index"])
    W_linX = np.asarray(inputs["W_linX"], dtype=np.float32)
    b_linX = np.asarray(inputs["b_linX"], dtype=np.float32)
    policy = np.asarray(inputs["policy"], dtype=np.float64)
    W_pred = np.asarray(inputs["W_pred"], dtype=np.float32)
    b_pred = np.asarray(inputs["b_pred"], dtype=np.float32)

    npc = n // ncores
    row = edge_index[0].astype(np.int64)
    col = edge_index[1].astype(np.int64)
    deg = np.bincount(col, minlength=n).astype(np.float64)
    dis = np.where(deg > 0, 1.0 / np.sqrt(np.maximum(deg, 1.0)), 0.0)
    shift = int(row.min())
    dst = row - shift                      # aggregation destination
    # per-dst factor is dis at the *unshifted* row id
    dis_sh = np.zeros(n, dtype=np.float64)
    hi = n - shift
    dis_sh[:hi] = dis[shift:]

    e = np.exp(policy[:2] - policy[:2].max())
    pp = e / e.sum()
    pp0, pp1 = float(pp[0]), float(pp[1])
    b_comb = pp0 + pp1                      # == 1.0, but don't rely on it

    # ---- per-core 1D bin packing of dst nodes
    cores = []
    for c in range(ncores):
        m = (dst >= c * npc) & (dst < (c + 1) * npc)
        e_dst = dst[m] - c * npc
        e_src = col[m]
        degc = np.bincount(e_dst, minlength=npc)
        bins = _pack_core(degc, CAP_EDGES, SUB_NODES)
        # canonical order: descending chunk count, so per-position maxima
        # across cores stay tight
        bins.sort(key=lambda b: -b[1])
        cores.append((e_dst, e_src, bins))

    max_bins = max(len(b) for _, _, b in cores)
    n_sub = -(-max_bins // GROUP_SUBS) * GROUP_SUBS
    n_grp = n_sub // GROUP_SUBS
    slots = n_sub * SUB_NODES               # table rows per core
    rows_tot = slots * ncores               # rows of the shared table
    assert rows_tot // 2 <= 32767, f"pair count {rows_tot//2} exceeds int16"

    # common per-position chunk counts (>=1 so every psum range is written)
    cnt = np.ones(n_sub, dtype=np.int64)
    for _, _, bins in cores:
        for i, (_, se) in enumerate(bins):
            cnt[i] = max(cnt[i], -(-se // CHUNK))
    off = np.zeros(n_sub + 1, dtype=np.int64)
    np.cumsum(cnt, out=off[1:])
    TC = int(off[n_sub])                    # total chunks per iteration

    # batches: uniform chunk ranges (groups may span several gather calls)
    batches = []                            # (ch0, nch)
    ch = 0
    while ch < TC:
        nch = min(BATCH_CHUNKS, TC - ch)
        batches.append((ch, nch))
        ch += nch
    max_nch = max(b[1] for b in batches)

    # slot assignment + per-core tensors
    slot_of_node = np.full(n, -1, dtype=np.int64)   # local slot within core
    for c, (e_dst, e_src, bins) in enumerate(cores):
        for bi, (nodes, _) in enumerate(bins):
            for k, v in enumerate(nodes):
                slot_of_node[c * npc + v] = bi * SUB_NODES + k
    core_of = np.arange(n) // npc
    # table rows split in two parts at a group boundary so the front part's
    # AllGather can launch mid-iteration: [c0 F | .. | c7 F | c0 B | .. c7 B]
    # front = whole table: one merged boundary AllGather measures faster than
    # any split variant (per-op overhead + contention inflation dominate)
    n_grp_f = n_grp
    slots_f = n_grp_f * GROUP_SUBS * SUB_NODES
    RF = ncores * slots_f
    s_loc = slot_of_node
    grow = np.where(s_loc < slots_f,
                    core_of * slots_f + s_loc,
                    RF + core_of * (slots - slots_f) + (s_loc - slots_f))

    per_core = []
    for c, (e_dst, e_src, bins) in enumerate(cores):
        dloc = slot_of_node[e_dst + c * npc]
        dst_bin = dloc // SUB_NODES
        dst_k = dloc % SUB_NODES
        # pair-gather: table rows gathered in PAIRS (256B fp8 descriptors);
        # per chunk an [even | odd] pair of 32-col blocks picks the row.
        S = np.zeros((TC, CHUNK, 2, SUB_NODES), dtype=np.float32)
        idx = np.zeros(TC * CHUNK, dtype=np.int64)  # pad -> pair 0 (S row 0)
        for bi in range(n_sub):
            sel = dst_bin == bi
            srcs = e_src[sel]
            dks = dst_k[sel]
            kk = len(srcs)
            assert kk <= cnt[bi] * CHUNK, (c, bi, kk)
            cbase = off[bi]
            gr = grow[srcs]
            for j in range(kk):
                ch = cbase + j // CHUNK
                S[ch, j % CHUNK, gr[j] % 2, dks[j]] = 1.0
            idx[cbase * CHUNK: cbase * CHUNK + kk] = gr // 2
        assert idx.min() >= 0 and idx.max() < rows_tot // 2

        def wrap_idx(ix):
            # index i consumed from [i % 16, i // 16]; replicate to 128 parts
            w = ix.reshape(-1, 16).T.astype(np.int16)      # [16, n/16]
            return np.tile(w, (8, 1))                       # [128, n/16]

        # slot-layout host arrays (x pre-transposed: [IN_C, slots])
        x_slot = np.zeros((slots, IN_C), dtype=np.float32)
        dis_slot = np.zeros(slots, dtype=np.float64)
        dsh_slot = np.zeros(slots, dtype=np.float64)
        nodes_c = np.arange(c * npc, (c + 1) * npc)
        sl = slot_of_node[nodes_c]
        x_slot[sl] = x[nodes_c]
        dis_slot[sl] = dis[nodes_c]
        dsh_slot[sl] = dis_sh[nodes_c]

        grp = lambda v: v.reshape(n_grp, 128).T.astype(np.float32)
        per_core.append({
            "x_slot": np.ascontiguousarray(x_slot.T),
            "S": np.ascontiguousarray(
                S.transpose(1, 0, 2, 3).reshape(CHUNK, -1)
                .astype(ml_dtypes.float8_e4m3)),             # [128, TC*64]
            "idx": wrap_idx(idx),
            "disg": grp(dis_slot),
            "dis2g": grp(dis_slot * dsh_slot),
            "ag": grp((pp1 / b_comb) * dsh_slot),
            "W_linX": W_linX,
            "bX": np.tile(b_linX[None, :], (128, 1)).astype(np.float32),
            "W_pred": (b_comb * W_pred).astype(np.float32),
            "bP": np.tile(b_pred[None, :], (128, 1)).astype(np.float32),
            "ident": np.eye(128, dtype=np.float32),
        })

    meta = dict(n=n, ncores=ncores, npc=npc, n_sub=n_sub, n_grp=n_grp,
                slots=slots, rows_tot=rows_tot, TC=TC,
                cnt=cnt, off=off, batches=batches, max_nch=max_nch,
                n_grp_f=n_grp_f, slots_f=slots_f, RF=RF,
                slot_of_node=slot_of_node)
    return meta, per_core


# ----------------------------------------------------------------------------
# Bass program
# ----------------------------------------------------------------------------

def _build_program(meta, iters=POWER1):
    import concourse.bacc as bacc
    import concourse.mybir as mybir
    from concourse import tile

    f32, f16, i16 = mybir.dt.float32, mybir.dt.float16, mybir.dt.int16
    f8 = mybir.dt.float8e4
    ADD, MULT = mybir.AluOpType.add, mybir.AluOpType.mult

    ncores = meta["ncores"]
    n_sub, n_grp = meta["n_sub"], meta["n_grp"]
    slots, rows_tot = meta["slots"], meta["rows_tot"]
    TC = meta["TC"]
    cnt, off = meta["cnt"], meta["off"]
    batches, max_nch = meta["batches"], meta["max_nch"]
    n_grp_f, slots_f, RF = meta["n_grp_f"], meta["slots_f"], meta["RF"]
    # first batch index by which every front-part group has been consumed
    bF = next(b for b, (ch0, nch) in enumerate(batches)
              if off[n_grp_f * GROUP_SUBS] <= ch0 + nch)

    nc = bacc.Bacc("TRN2", target_bir_lowering=False, debug=False,
                   enable_asserts=False, num_devices=ncores,
                   num_swdge_queues=NQUEUES,
                   dynamic_dma_scratch_size=32768)

    x_slot_h = nc.dram_tensor("x_slot", [IN_C, slots], f32, kind="ExternalInput")
    S_h = nc.dram_tensor("S", [CHUNK, TC * 2 * SUB_NODES], f8,
                         kind="ExternalInput")
    idx_h = nc.dram_tensor("idx", [128, TC * CHUNK // 16], i16,
                           kind="ExternalInput")
    disg_h = nc.dram_tensor("disg", [128, n_grp], f32, kind="ExternalInput")
    dis2g_h = nc.dram_tensor("dis2g", [128, n_grp], f32, kind="ExternalInput")
    ag_h = nc.dram_tensor("ag", [128, n_grp], f32, kind="ExternalInput")
    W_h = nc.dram_tensor("W_linX", [IN_C, HID], f32, kind="ExternalInput")
    bX_h = nc.dram_tensor("bX", [128, HID], f32, kind="ExternalInput")
    Wp_h = nc.dram_tensor("W_pred", [HID, OUT_C], f32, kind="ExternalInput")
    bP_h = nc.dram_tensor("bP", [128, OUT_C], f32, kind="ExternalInput")
    id_h = nc.dram_tensor("ident", [128, 128], f32, kind="ExternalInput")

    # ping-pong shared table (fp8, viewed as row PAIRS for 256B-elem gathers)
    tab = [nc.dram_tensor(f"tab{p}", [rows_tot // 2, 2 * HID], f8,
                          addr_space="Shared") for p in range(2)]
    shardF = [nc.dram_tensor(f"shardF{p}", [slots_f, HID], f8)
              for p in range(2)]
    shardB = ([nc.dram_tensor(f"shardB{p}", [slots - slots_f, HID], f8)
               for p in range(2)] if slots > slots_f else None)
    out_h = nc.dram_tensor("out", [slots, OUT_C], f32, kind="ExternalOutput")

    rg = [list(range(ncores))]

    def allgather(src, dst_ap):
        nc.gpsimd.collective_compute(
            "AllGather", mybir.AluOpType.bypass, replica_groups=rg,
            ins=[src.ap().opt()], outs=[dst_ap.opt()])

    with tile.TileContext(nc, num_cores=ncores) as tc:
        import contextlib
        with contextlib.ExitStack() as ctx:
            cpool = ctx.enter_context(tc.tile_pool(name="const", bufs=1))
            wpool = ctx.enter_context(tc.tile_pool(name="work", bufs=2))
            xpool = ctx.enter_context(tc.tile_pool(name="xload", bufs=1))
            gpool = ctx.enter_context(tc.tile_pool(name="gath", bufs=16))
            spool = ctx.enter_context(tc.tile_pool(name="stage", bufs=6))
            ppool = ctx.enter_context(
                tc.tile_pool(name="psum", bufs=6, space="PSUM"))
            tpool = ctx.enter_context(
                tc.tile_pool(name="psum2", bufs=1, space="PSUM"))

            # persistent SBUF
            S_sb = cpool.tile([CHUNK, TC * 2 * SUB_NODES], f8)
            nc.sync.dma_start(S_sb[:, :], S_h[:, :])
            idx_sb = cpool.tile([128, TC * CHUNK // 16], i16)
            nc.sync.dma_start(idx_sb[:, :], idx_h[:, :])
            disg = cpool.tile([128, n_grp], f32)
            nc.sync.dma_start(disg[:, :], disg_h[:, :])
            dis2g = cpool.tile([128, n_grp], f32)
            nc.sync.dma_start(dis2g[:, :], dis2g_h[:, :])
            ag = cpool.tile([128, n_grp], f32)
            nc.sync.dma_start(ag[:, :], ag_h[:, :])
            W_sb = cpool.tile([IN_C, HID], f32)
            nc.sync.dma_start(W_sb[:, :], W_h[:, :])
            bX_sb = cpool.tile([128, HID], f32)
            nc.sync.dma_start(bX_sb[:, :], bX_h[:, :])
            Wp_sb = cpool.tile([HID, OUT_C], f32)
            nc.sync.dma_start(Wp_sb[:, :], Wp_h[:, :])
            bP_sb = cpool.tile([128, OUT_C], f32)
            nc.sync.dma_start(bP_sb[:, :], bP_h[:, :])
            ident = cpool.tile([128, 128], f32)
            nc.sync.dma_start(ident[:, :], id_h[:, :])
            xX_sb = cpool.tile([128, n_grp * HID], f32)     # computed below
            dxX_sb = cpool.tile([128, n_grp * HID], f32)    # disg * xX

            # ---- prologue: xX = x @ W + b; T0 = dis * xX -> shard
            xT_sb = xpool.tile([128, n_grp * 128], f32, tag="xT")
            nc.sync.dma_start(xT_sb[:, :], x_slot_h[:, :])
            for g in range(n_grp):
                rows = slice(g * 128, (g + 1) * 128)
                gc = slice(g * HID, (g + 1) * HID)
                mm_ps = ppool.tile([128, HID], f32, tag="ps")
                nc.tensor.matmul(mm_ps[:, :],
                                 xT_sb[:, g * 128:(g + 1) * 128],
                                 W_sb[:, :], start=True, stop=True)
                nc.vector.tensor_tensor(xX_sb[:, gc], mm_ps[:, :],
                                        bX_sb[:, :], op=ADD)
                nc.vector.tensor_scalar_mul(dxX_sb[:, gc], xX_sb[:, gc],
                                            disg[:, g:g + 1])
                stage = spool.tile([128, HID], f8, tag="stage")
                nc.vector.tensor_copy(stage[:, :], dxX_sb[:, gc])
                if g < n_grp_f:
                    nc.sync.dma_start(shardF[0][rows, :], stage[:, :])
                else:
                    rb = slice((g - n_grp_f) * 128, (g - n_grp_f + 1) * 128)
                    nc.sync.dma_start(shardB[0][rb, :], stage[:, :])
                if g == n_grp_f - 1:
                    allgather(shardF[0], tab[0][0:RF // 2, :])

            # ---- 8 SpMM iterations.
            # The FRONT part's AllGather for iteration t+1 launches
            # mid-iteration t (contended with gather traffic but hidden);
            # only the BACK part's AllGather sits in the boundary quiet
            # window (G#0 of the next iteration head-of-line parks the
            # in-order Pool sequencer until it lands - concurrent gathers
            # would inflate the collective ~3x).
            for t in range(iters):
                p = t % 2
                last = t == iters - 1
                if t > 0:
                    allgather(shardF[p], tab[p][0:RF // 2, :])
                if shardB is not None:
                    allgather(shardB[p], tab[p][RF // 2:rows_tot // 2, :])
                tiles = {}
                gdone = 0
                for b, (ch0, nch) in enumerate(batches):
                    nidx = nch * CHUNK
                    mg = gpool.tile([128, max_nch * CHUNK * 2], f8, tag="mg")
                    tiles[b] = mg
                    nc.gpsimd.dma_gather(
                        mg[:, :nidx * 2].rearrange("p (c e) -> p c e",
                                                   e=2 * HID),
                        tab[p][0:rows_tot // 2, :],
                        idx_sb[:, ch0 * (CHUNK // 16):
                               (ch0 + nch) * (CHUNK // 16)],
                        num_idxs=nidx, num_idxs_reg=nidx,
                        elem_size=2 * HID, single_packet=True,
                        queue_num=b % NQUEUES)
                    end = ch0 + nch
                    while (gdone < n_grp
                           and off[(gdone + 1) * GROUP_SUBS] <= end):
                        g = gdone
                        gdone += 1
                        gc = slice(g * HID, (g + 1) * HID)
                        ps = ppool.tile([128, HID], f32, tag="ps")
                        for j in range(GROUP_SUBS):
                            sb = g * GROUP_SUBS + j
                            prange = slice(32 * j, 32 * j + 32)
                            tpos = (0, 32 * j)
                            nch_b = int(cnt[sb])
                            for k in range(nch_b):
                                cg = int(off[sb]) + k          # global chunk
                                bb = cg // BATCH_CHUNKS        # owning batch
                                q = cg - bb * BATCH_CHUNKS
                                mgb = tiles[bb]
                                for par in range(2):
                                    nc.tensor.matmul(
                                        ps[prange, :],
                                        S_sb[:, cg * 64 + par * 32:
                                             cg * 64 + par * 32 + 32],
                                        mgb[:, q * 256 + par * HID:
                                            q * 256 + (par + 1) * HID],
                                        start=(k == 0 and par == 0),
                                        stop=(k == nch_b - 1 and par == 1),
                                        tile_position=tpos)
                        if not last:
                            stage = spool.tile([128, HID], f8, tag="stage")
                            nc.vector.scalar_tensor_tensor(
                                stage[:, :], ps[:, :], dis2g[:, g:g + 1],
                                dxX_sb[:, gc], op0=MULT, op1=ADD)
                            pn = (t + 1) % 2
                            if g < n_grp_f:
                                rows = slice(g * 128, (g + 1) * 128)
                                nc.sync.dma_start(shardF[pn][rows, :],
                                                  stage[:, :])
                            else:
                                rb = slice((g - n_grp_f) * 128,
                                           (g - n_grp_f + 1) * 128)
                                nc.sync.dma_start(shardB[pn][rb, :],
                                                  stage[:, :])
                        else:
                            u_t = wpool.tile([128, HID], f32, tag="t1")
                            nc.vector.scalar_tensor_tensor(
                                u_t[:, :], ps[:, :], ag[:, g:g + 1],
                                xX_sb[:, gc], op0=MULT, op1=ADD)
                            nc.vector.tensor_scalar_max(u_t[:, :], u_t[:, :],
                                                        0.0)
                            tp_ps = tpool.tile([128, 128], f32, tag="tp")
                            nc.tensor.transpose(tp_ps[:, :], u_t[:, :],
                                                ident[:, :])
                            uT_sb = wpool.tile([128, 128], f32, tag="xT")
                            nc.vector.tensor_copy(uT_sb[:, :], tp_ps[:, :])
                            o_ps = tpool.tile([128, OUT_C], f32, tag="mm2")
                            nc.tensor.matmul(o_ps[:, :], uT_sb[:, :],
                                             Wp_sb[:, :], start=True, stop=True)
                            o_sb = spool.tile([128, OUT_C], f32, tag="osb")
                            nc.vector.tensor_tensor(o_sb[:, :], o_ps[:, :],
                                                    bP_sb[:, :], op=ADD)
                            rows = slice(g * 128, (g + 1) * 128)
                            nc.sync.dma_start(out_h[rows, :], o_sb[:, :])

    nc.compile()
    return nc


# ----------------------------------------------------------------------------
# Runner
# ----------------------------------------------------------------------------

def _run(inputs, n=N, ncores=NCORES, trace=False, use_sim=False, iters=POWER1):
    meta, per_core = _preprocess(inputs, n=n, ncores=ncores)
    nc = _build_program(meta, iters=iters)
    in_maps = [dict(pc) for pc in per_core]

    if use_sim:
        from concourse.bass_interp import MultiCoreSim
        sim = MultiCoreSim(nc, num_cores=ncores)
        for c in range(ncores):
            for k, v in in_maps[c].items():
                sim.cores[c].tensor(k)[:] = v
        sim.simulate(check_with_hw=False)
        results = [{"out": np.array(sim.cores[c].tensor("out"))}
                   for c in range(ncores)]
        bres = None
    else:
        from concourse.bass_utils import run_bass_kernel_spmd
        bres = run_bass_kernel_spmd(nc, in_maps, core_ids=list(range(ncores)),
                                    trace=trace)
        results = bres.results

    # unshard: slots -> nodes
    npc = meta["npc"]
    son = meta["slot_of_node"]
    out = np.zeros((n, OUT_C), dtype=np.float32)
    for c in range(ncores):
        nodes = np.arange(c * npc, (c + 1) * npc)
        out[nodes] = results[c]["out"][son[nodes]]
    return out, bres


def kernel(**inputs) -> np.ndarray:
    # Run twice and compare: guards against rare transient device faults
    # (observed once after an unrecoverable-NRT event on a shared terminal).
    out1, _ = _run(inputs)
    out2, _ = _run(inputs)
    if np.allclose(out1, out2, rtol=0, atol=1e-4):
        return out1
    out3, _ = _run(inputs)
    if np.allclose(out1, out3, rtol=0, atol=1e-4):
        return out1
    return out2 if np.allclose(out2, out3, rtol=0, atol=1e-4) else out3


# revision 46
# speedup vs baseline: 1.0600x; 1.0600x over previous
"""GCN-style 8-step SpMM power iteration on 8 Trainium2 NeuronCores.

Math (reference):
    deg = segment_sum(1, col); dis = rsqrt(max(deg,1)) where deg>0 else 0
    norm_e = dis[row_e] * dis[col_e];  row' = row - row.min()
    xX = x @ W_linX + b_linX
    hX_{t+1}[v] = sum_{e: row'_e = v} norm_e * hX_t[col_e] + xX[v]   (8 times)
    out = relu(pp0*xX + pp1*hX_8) @ W_pred + b_pred

Key algebraic trick: norm factorizes per-edge into src/dst node factors, so we
keep the node table pre-scaled: T = dis ⊙ hX. Then one step is
    S[v]   = sum_{e->v} T[col_e]            (pure gather + segment-sum, no
                                             per-edge arithmetic at all)
    T'[v]  = (dis*dis_sh)[v]*S[v] + (dis*xX)[v]   (one DVE op per group)

Distribution: nodes dst-sharded over 8 cores; edges partitioned by dst core.
Nodes are relabeled into "slots": 32-slot sub-blocks (bins) packed so each
bin's in-edges fit a variable number of 128-edge chunks; 4 bins = one PSUM
group of 128 dst rows. Per iteration each core stages its T' shard (fp8) and
one AllGather rebuilds the full table on every core at the iteration
boundary. The boundary AG runs in a quiet window (the first gather of the
next iteration head-of-line parks the in-order Pool sequencer until the AG
lands) because concurrent SWDGE gather traffic inflates collective latency
~3x (small random reads keep the DMA engines latency-saturated).

The table is fp8 (e4m3); rows are gathered in PAIRS (256B descriptors - the
dma_gather minimum) and a per-chunk [even|odd] pair of 32-col fp8 selection
blocks picks the right row of each gathered pair on the TensorEngine,
accumulating fp32 in PSUM. Descriptor generation on the 4 SWDGE queues
(~2ns/idx aggregate) and the DMA engines' random-read latency are the
co-limiting resources; total gathered indices are minimized by per-bin
variable chunk counts (cross-core aligned by sorted position).
"""

import numpy as np

# problem shape (hardcoded per the task contract)
N = 50000
E = 800000
IN_C = 128
HID = 128
OUT_C = 40
POWER1 = 8

NCORES = 8
SUB_NODES = 32          # destination slots per sub-block (= matmul M)
CHUNK = 128             # edges per chunk (= matmul K)
GROUP_SUBS = 4          # sub-blocks per psum group ([128,128] psum tile)
CAP_EDGES = 512         # max in-edges per bin (4 chunks)
BATCH_CHUNKS = 8        # max chunks per gather call (1024 idx)
NQUEUES = 4             # SWDGE queues used for the gathers


# ----------------------------------------------------------------------------
# Host-side preprocessing
# ----------------------------------------------------------------------------

def _pack_core(deg, cap, sub_nodes):
    """1D best-fit-decreasing packing of nodes into bins (<=sub_nodes nodes,
    <=cap edges)."""
    order = np.argsort(-deg, kind="stable")
    bins = []        # (node_list, sum_edges)
    for v in order:
        a = int(deg[v])
        best, best_slack = -1, None
        for i, (nodes, sa) in enumerate(bins):
            if len(nodes) < sub_nodes and sa + a <= cap:
                slack = cap - sa - a
                if best_slack is None or slack < best_slack:
                    best, best_slack = i, slack
        if best < 0:
            bins.append(([v], a))
        else:
            nodes, sa = bins[best]
            nodes.append(v)
            bins[best] = (nodes, sa + a)
    return bins


def _preprocess(inputs, n=N, ncores=NCORES):
    import ml_dtypes
    x = np.asarray(inputs["x"], dtype=np.float32)
    edge_index = np.asarray(inputs["edge_index"])
    W_linX = np.asarray(inputs["W_linX"], dtype=np.float32)
    b_linX = np.asarray(inputs["b_linX"], dtype=np.float32)
    policy = np.asarray(inputs["policy"], dtype=np.float64)
    W_pred = np.asarray(inputs["W_pred"], dtype=np.float32)
    b_pred = np.asarray(inputs["b_pred"], dtype=np.float32)

    npc = n // ncores
    row = edge_index[0].astype(np.int64)
    col = edge_index[1].astype(np.int64)
    deg = np.bincount(col, minlength=n).astype(np.float64)
    dis = np.where(deg > 0, 1.0 / np.sqrt(np.maximum(deg, 1.0)), 0.0)
    shift = int(row.min())
    dst = row - shift                      # aggregation destination
    # per-dst factor is dis at the *unshifted* row id
    dis_sh = np.zeros(n, dtype=np.float64)
    hi = n - shift
    dis_sh[:hi] = dis[shift:]

    e = np.exp(policy[:2] - policy[:2].max())
    pp = e / e.sum()
    pp0, pp1 = float(pp[0]), float(pp[1])
    b_comb = pp0 + pp1                      # == 1.0, but don't rely on it

    # ---- per-core 1D bin packing of dst nodes
    cores = []
    for c in range(ncores):
        m = (dst >= c * npc) & (dst < (c + 1) * npc)
        e_dst = dst[m] - c * npc
        e_src = col[m]
        degc = np.bincount(e_dst, minlength=npc)
        bins = _pack_core(degc, CAP_EDGES, SUB_NODES)
        # canonical order: descending chunk count, so per-position maxima
        # across cores stay tight
        bins.sort(key=lambda b: -b[1])
        cores.append((e_dst, e_src, bins))

    max_bins = max(len(b) for _, _, b in cores)
    n_sub = -(-max_bins // GROUP_SUBS) * GROUP_SUBS
    n_grp = n_sub // GROUP_SUBS
    slots = n_sub * SUB_NODES               # table rows per core
    rows_tot = slots * ncores               # rows of the shared table
    assert rows_tot // 2 <= 32767, f"pair count {rows_tot//2} exceeds int16"

    # common per-position chunk counts (>=1 so every psum range is written)
    cnt = np.ones(n_sub, dtype=np.int64)
    for _, _, bins in cores:
        for i, (_, se) in enumerate(bins):
            cnt[i] = max(cnt[i], -(-se // CHUNK))
    off = np.zeros(n_sub + 1, dtype=np.int64)
    np.cumsum(cnt, out=off[1:])
    TC = int(off[n_sub])                    # total chunks per iteration

    # batches: uniform chunk ranges (groups may span several gather calls)
    batches = []                            # (ch0, nch)
    ch = 0
    while ch < TC:
        nch = min(BATCH_CHUNKS, TC - ch)
        batches.append((ch, nch))
        ch += nch
    max_nch = max(b[1] for b in batches)

    # slot assignment + per-core tensors
    slot_of_node = np.full(n, -1, dtype=np.int64)   # local slot within core
    for c, (e_dst, e_src, bins) in enumerate(cores):
        for bi, (nodes, _) in enumerate(bins):
            for k, v in enumerate(nodes):
                slot_of_node[c * npc + v] = bi * SUB_NODES + k
    core_of = np.arange(n) // npc
    # table rows split in two parts at a group boundary so the front part's
    # AllGather can launch mid-iteration: [c0 F | .. | c7 F | c0 B | .. c7 B]
    # front ~63%, back ~37%: both AllGathers run in the boundary quiet
    # window; measured faster than a single merged AG or mid-iteration
    # launches under single_packet coalescing
    n_grp_f = int(round(n_grp * 0.63))
    slots_f = n_grp_f * GROUP_SUBS * SUB_NODES
    RF = ncores * slots_f
    s_loc = slot_of_node
    grow = np.where(s_loc < slots_f,
                    core_of * slots_f + s_loc,
                    RF + core_of * (slots - slots_f) + (s_loc - slots_f))

    per_core = []
    for c, (e_dst, e_src, bins) in enumerate(cores):
        dloc = slot_of_node[e_dst + c * npc]
        dst_bin = dloc // SUB_NODES
        dst_k = dloc % SUB_NODES
        # pair-gather: table rows gathered in PAIRS (256B fp8 descriptors);
        # per chunk an [even | odd] pair of 32-col blocks picks the row.
        S = np.zeros((TC, CHUNK, 2, SUB_NODES), dtype=np.float32)
        idx = np.zeros(TC * CHUNK, dtype=np.int64)  # pad -> pair 0 (S row 0)
        for bi in range(n_sub):
            sel = dst_bin == bi
            srcs = e_src[sel]
            dks = dst_k[sel]
            kk = len(srcs)
            assert kk <= cnt[bi] * CHUNK, (c, bi, kk)
            cbase = off[bi]
            gr = grow[srcs]
            for j in range(kk):
                ch = cbase + j // CHUNK
                S[ch, j % CHUNK, gr[j] % 2, dks[j]] = 1.0
            idx[cbase * CHUNK: cbase * CHUNK + kk] = gr // 2
        assert idx.min() >= 0 and idx.max() < rows_tot // 2

        def wrap_idx(ix):
            # index i consumed from [i % 16, i // 16]; replicate to 128 parts
            w = ix.reshape(-1, 16).T.astype(np.int16)      # [16, n/16]
            return np.tile(w, (8, 1))                       # [128, n/16]

        # slot-layout host arrays (x pre-transposed: [IN_C, slots])
        x_slot = np.zeros((slots, IN_C), dtype=np.float32)
        dis_slot = np.zeros(slots, dtype=np.float64)
        dsh_slot = np.zeros(slots, dtype=np.float64)
        nodes_c = np.arange(c * npc, (c + 1) * npc)
        sl = slot_of_node[nodes_c]
        x_slot[sl] = x[nodes_c]
        dis_slot[sl] = dis[nodes_c]
        dsh_slot[sl] = dis_sh[nodes_c]

        grp = lambda v: v.reshape(n_grp, 128).T.astype(np.float32)
        per_core.append({
            "x_slot": np.ascontiguousarray(x_slot.T),
            "S": np.ascontiguousarray(
                S.transpose(1, 0, 2, 3).reshape(CHUNK, -1)
                .astype(ml_dtypes.float8_e4m3)),             # [128, TC*64]
            "idx": wrap_idx(idx),
            "disg": grp(dis_slot),
            "dis2g": grp(dis_slot * dsh_slot),
            "ag": grp((pp1 / b_comb) * dsh_slot),
            "W_linX": W_linX,
            "bX": np.tile(b_linX[None, :], (128, 1)).astype(np.float32),
            "W_pred": (b_comb * W_pred).astype(np.float32),
            "bP": np.tile(b_pred[None, :], (128, 1)).astype(np.float32),
            "ident": np.eye(128, dtype=np.float32),
        })

    meta = dict(n=n, ncores=ncores, npc=npc, n_sub=n_sub, n_grp=n_grp,
                slots=slots, rows_tot=rows_tot, TC=TC,
                cnt=cnt, off=off, batches=batches, max_nch=max_nch,
                n_grp_f=n_grp_f, slots_f=slots_f, RF=RF,
                slot_of_node=slot_of_node)
    return meta, per_core


# ----------------------------------------------------------------------------
# Bass program
# ----------------------------------------------------------------------------

def _build_program(meta, iters=POWER1):
    import concourse.bacc as bacc
    import concourse.mybir as mybir
    from concourse import tile

    f32, f16, i16 = mybir.dt.float32, mybir.dt.float16, mybir.dt.int16
    f8 = mybir.dt.float8e4
    ADD, MULT = mybir.AluOpType.add, mybir.AluOpType.mult

    ncores = meta["ncores"]
    n_sub, n_grp = meta["n_sub"], meta["n_grp"]
    slots, rows_tot = meta["slots"], meta["rows_tot"]
    TC = meta["TC"]
    cnt, off = meta["cnt"], meta["off"]
    batches, max_nch = meta["batches"], meta["max_nch"]
    n_grp_f, slots_f, RF = meta["n_grp_f"], meta["slots_f"], meta["RF"]
    # first batch index by which every front-part group has been consumed
    bF = next(b for b, (ch0, nch) in enumerate(batches)
              if off[n_grp_f * GROUP_SUBS] <= ch0 + nch)

    nc = bacc.Bacc("TRN2", target_bir_lowering=False, debug=False,
                   enable_asserts=False, num_devices=ncores,
                   num_swdge_queues=NQUEUES,
                   dynamic_dma_scratch_size=32768)

    x_slot_h = nc.dram_tensor("x_slot", [IN_C, slots], f32, kind="ExternalInput")
    S_h = nc.dram_tensor("S", [CHUNK, TC * 2 * SUB_NODES], f8,
                         kind="ExternalInput")
    idx_h = nc.dram_tensor("idx", [128, TC * CHUNK // 16], i16,
                           kind="ExternalInput")
    disg_h = nc.dram_tensor("disg", [128, n_grp], f32, kind="ExternalInput")
    dis2g_h = nc.dram_tensor("dis2g", [128, n_grp], f32, kind="ExternalInput")
    ag_h = nc.dram_tensor("ag", [128, n_grp], f32, kind="ExternalInput")
    W_h = nc.dram_tensor("W_linX", [IN_C, HID], f32, kind="ExternalInput")
    bX_h = nc.dram_tensor("bX", [128, HID], f32, kind="ExternalInput")
    Wp_h = nc.dram_tensor("W_pred", [HID, OUT_C], f32, kind="ExternalInput")
    bP_h = nc.dram_tensor("bP", [128, OUT_C], f32, kind="ExternalInput")
    id_h = nc.dram_tensor("ident", [128, 128], f32, kind="ExternalInput")

    # ping-pong shared table (fp8, viewed as row PAIRS for 256B-elem gathers)
    tab = [nc.dram_tensor(f"tab{p}", [rows_tot // 2, 2 * HID], f8,
                          addr_space="Shared") for p in range(2)]
    shardF = [nc.dram_tensor(f"shardF{p}", [slots_f, HID], f8)
              for p in range(2)]
    shardB = ([nc.dram_tensor(f"shardB{p}", [slots - slots_f, HID], f8)
               for p in range(2)] if slots > slots_f else None)
    out_h = nc.dram_tensor("out", [slots, OUT_C], f32, kind="ExternalOutput")

    rg = [list(range(ncores))]

    def allgather(src, dst_ap):
        nc.gpsimd.collective_compute(
            "AllGather", mybir.AluOpType.bypass, replica_groups=rg,
            ins=[src.ap().opt()], outs=[dst_ap.opt()])

    with tile.TileContext(nc, num_cores=ncores) as tc:
        import contextlib
        with contextlib.ExitStack() as ctx:
            cpool = ctx.enter_context(tc.tile_pool(name="const", bufs=1))
            wpool = ctx.enter_context(tc.tile_pool(name="work", bufs=2))
            xpool = ctx.enter_context(tc.tile_pool(name="xload", bufs=1))
            gpool = ctx.enter_context(tc.tile_pool(name="gath", bufs=16))
            spool = ctx.enter_context(tc.tile_pool(name="stage", bufs=6))
            ppool = ctx.enter_context(
                tc.tile_pool(name="psum", bufs=6, space="PSUM"))
            tpool = ctx.enter_context(
                tc.tile_pool(name="psum2", bufs=1, space="PSUM"))

            # persistent SBUF
            S_sb = cpool.tile([CHUNK, TC * 2 * SUB_NODES], f8)
            nc.sync.dma_start(S_sb[:, :], S_h[:, :])
            idx_sb = cpool.tile([128, TC * CHUNK // 16], i16)
            nc.sync.dma_start(idx_sb[:, :], idx_h[:, :])
            disg = cpool.tile([128, n_grp], f32)
            nc.sync.dma_start(disg[:, :], disg_h[:, :])
            dis2g = cpool.tile([128, n_grp], f32)
            nc.sync.dma_start(dis2g[:, :], dis2g_h[:, :])
            ag = cpool.tile([128, n_grp], f32)
            nc.sync.dma_start(ag[:, :], ag_h[:, :])
            W_sb = cpool.tile([IN_C, HID], f32)
            nc.sync.dma_start(W_sb[:, :], W_h[:, :])
            bX_sb = cpool.tile([128, HID], f32)
            nc.sync.dma_start(bX_sb[:, :], bX_h[:, :])
            Wp_sb = cpool.tile([HID, OUT_C], f32)
            nc.sync.dma_start(Wp_sb[:, :], Wp_h[:, :])
            bP_sb = cpool.tile([128, OUT_C], f32)
            nc.sync.dma_start(bP_sb[:, :], bP_h[:, :])
            ident = cpool.tile([128, 128], f32)
            nc.sync.dma_start(ident[:, :], id_h[:, :])
            xX_sb = cpool.tile([128, n_grp * HID], f32)     # computed below
            dxX_sb = cpool.tile([128, n_grp * HID], f32)    # disg * xX

            # ---- prologue: xX = x @ W + b; T0 = dis * xX -> shard
            xT_sb = xpool.tile([128, n_grp * 128], f32, tag="xT")
            nc.sync.dma_start(xT_sb[:, :], x_slot_h[:, :])
            for g in range(n_grp):
                rows = slice(g * 128, (g + 1) * 128)
                gc = slice(g * HID, (g + 1) * HID)
                mm_ps = ppool.tile([128, HID], f32, tag="ps")
                nc.tensor.matmul(mm_ps[:, :],
                                 xT_sb[:, g * 128:(g + 1) * 128],
                                 W_sb[:, :], start=True, stop=True)
                nc.vector.tensor_tensor(xX_sb[:, gc], mm_ps[:, :],
                                        bX_sb[:, :], op=ADD)
                nc.vector.tensor_scalar_mul(dxX_sb[:, gc], xX_sb[:, gc],
                                            disg[:, g:g + 1])
                stage = spool.tile([128, HID], f8, tag="stage")
                nc.vector.tensor_copy(stage[:, :], dxX_sb[:, gc])
                if g < n_grp_f:
                    nc.sync.dma_start(shardF[0][rows, :], stage[:, :])
                else:
                    rb = slice((g - n_grp_f) * 128, (g - n_grp_f + 1) * 128)
                    nc.sync.dma_start(shardB[0][rb, :], stage[:, :])
                if g == n_grp_f - 1:
                    allgather(shardF[0], tab[0][0:RF // 2, :])

            # ---- 8 SpMM iterations.
            # The FRONT part's AllGather for iteration t+1 launches
            # mid-iteration t (contended with gather traffic but hidden);
            # only the BACK part's AllGather sits in the boundary quiet
            # window (G#0 of the next iteration head-of-line parks the
            # in-order Pool sequencer until it lands - concurrent gathers
            # would inflate the collective ~3x).
            for t in range(iters):
                p = t % 2
                last = t == iters - 1
                if t > 0:
                    allgather(shardF[p], tab[p][0:RF // 2, :])
                if shardB is not None:
                    allgather(shardB[p], tab[p][RF // 2:rows_tot // 2, :])
                tiles = {}
                gdone = 0
                for b, (ch0, nch) in enumerate(batches):
                    nidx = nch * CHUNK
                    mg = gpool.tile([128, max_nch * CHUNK * 2], f8, tag="mg")
                    tiles[b] = mg
                    nc.gpsimd.dma_gather(
                        mg[:, :nidx * 2].rearrange("p (c e) -> p c e",
                                                   e=2 * HID),
                        tab[p][0:rows_tot // 2, :],
                        idx_sb[:, ch0 * (CHUNK // 16):
                               (ch0 + nch) * (CHUNK // 16)],
                        num_idxs=nidx, num_idxs_reg=nidx,
                        elem_size=2 * HID, single_packet=True,
                        queue_num=b % NQUEUES)
                    end = ch0 + nch
                    while (gdone < n_grp
                           and off[(gdone + 1) * GROUP_SUBS] <= end):
                        g = gdone
                        gdone += 1
                        gc = slice(g * HID, (g + 1) * HID)
                        ps = ppool.tile([128, HID], f32, tag="ps")
                        for j in range(GROUP_SUBS):
                            sb = g * GROUP_SUBS + j
                            prange = slice(32 * j, 32 * j + 32)
                            tpos = (0, 32 * j)
                            nch_b = int(cnt[sb])
                            for k in range(nch_b):
                                cg = int(off[sb]) + k          # global chunk
                                bb = cg // BATCH_CHUNKS        # owning batch
                                q = cg - bb * BATCH_CHUNKS
                                mgb = tiles[bb]
                                for par in range(2):
                                    nc.tensor.matmul(
                                        ps[prange, :],
                                        S_sb[:, cg * 64 + par * 32:
                                             cg * 64 + par * 32 + 32],
                                        mgb[:, q * 256 + par * HID:
                                            q * 256 + (par + 1) * HID],
                                        start=(k == 0 and par == 0),
                                        stop=(k == nch_b - 1 and par == 1),
                                        tile_position=tpos)
                        if not last:
                            stage = spool.tile([128, HID], f8, tag="stage")
                            nc.vector.scalar_tensor_tensor(
                                stage[:, :], ps[:, :], dis2g[:, g:g + 1],
                                dxX_sb[:, gc], op0=MULT, op1=ADD)
                            pn = (t + 1) % 2
                            if g < n_grp_f:
                                rows = slice(g * 128, (g + 1) * 128)
                                nc.sync.dma_start(shardF[pn][rows, :],
                                                  stage[:, :])
                            else:
                                rb = slice((g - n_grp_f) * 128,
                                           (g - n_grp_f + 1) * 128)
                                nc.sync.dma_start(shardB[pn][rb, :],
                                                  stage[:, :])
                        else:
                            u_t = wpool.tile([128, HID], f32, tag="t1")
                            nc.vector.scalar_tensor_tensor(
                                u_t[:, :], ps[:, :], ag[:, g:g + 1],
                                xX_sb[:, gc], op0=MULT, op1=ADD)
                            nc.vector.tensor_scalar_max(u_t[:, :], u_t[:, :],
                                                        0.0)
                            tp_ps = tpool.tile([128, 128], f32, tag="tp")
                            nc.tensor.transpose(tp_ps[:, :], u_t[:, :],
                                                ident[:, :])
                            uT_sb = wpool.tile([128, 128], f32, tag="xT")
                            nc.vector.tensor_copy(uT_sb[:, :], tp_ps[:, :])
                            o_ps = tpool.tile([128, OUT_C], f32, tag="mm2")
                            nc.tensor.matmul(o_ps[:, :], uT_sb[:, :],
                                             Wp_sb[:, :], start=True, stop=True)
                            o_sb = spool.tile([128, OUT_C], f32, tag="osb")
                            nc.vector.tensor_tensor(o_sb[:, :], o_ps[:, :],
                                                    bP_sb[:, :], op=ADD)
                            rows = slice(g * 128, (g + 1) * 128)
                            nc.sync.dma_start(out_h[rows, :], o_sb[:, :])

    nc.compile()
    return nc


# ----------------------------------------------------------------------------
# Runner
# ----------------------------------------------------------------------------

def _run(inputs, n=N, ncores=NCORES, trace=False, use_sim=False, iters=POWER1):
    meta, per_core = _preprocess(inputs, n=n, ncores=ncores)
    nc = _build_program(meta, iters=iters)
    in_maps = [dict(pc) for pc in per_core]

    if use_sim:
        from concourse.bass_interp import MultiCoreSim
        sim = MultiCoreSim(nc, num_cores=ncores)
        for c in range(ncores):
            for k, v in in_maps[c].items():
                sim.cores[c].tensor(k)[:] = v
        sim.simulate(check_with_hw=False)
        results = [{"out": np.array(sim.cores[c].tensor("out"))}
                   for c in range(ncores)]
        bres = None
    else:
        from concourse.bass_utils import run_bass_kernel_spmd
        bres = run_bass_kernel_spmd(nc, in_maps, core_ids=list(range(ncores)),
                                    trace=trace)
        results = bres.results

    # unshard: slots -> nodes
    npc = meta["npc"]
    son = meta["slot_of_node"]
    out = np.zeros((n, OUT_C), dtype=np.float32)
    for c in range(ncores):
        nodes = np.arange(c * npc, (c + 1) * npc)
        out[nodes] = results[c]["out"][son[nodes]]
    return out, bres


def kernel(**inputs) -> np.ndarray:
    # Run twice and compare: guards against rare transient device faults
    # (observed once after an unrecoverable-NRT event on a shared terminal).
    out1, _ = _run(inputs)
    out2, _ = _run(inputs)
    if np.allclose(out1, out2, rtol=0, atol=1e-4):
        return out1
    out3, _ = _run(inputs)
    if np.allclose(out1, out3, rtol=0, atol=1e-4):
        return out1
    return out2 if np.allclose(out2, out3, rtol=0, atol=1e-4) else out3


# revision 48
# speedup vs baseline: 1.0808x; 1.0196x over previous
"""GCN-style 8-step SpMM power iteration on 8 Trainium2 NeuronCores.

Math (reference):
    deg = segment_sum(1, col); dis = rsqrt(max(deg,1)) where deg>0 else 0
    norm_e = dis[row_e] * dis[col_e];  row' = row - row.min()
    xX = x @ W_linX + b_linX
    hX_{t+1}[v] = sum_{e: row'_e = v} norm_e * hX_t[col_e] + xX[v]   (8 times)
    out = relu(pp0*xX + pp1*hX_8) @ W_pred + b_pred

Key algebraic trick: norm factorizes per-edge into src/dst node factors, so we
keep the node table pre-scaled: T = dis ⊙ hX. Then one step is
    S[v]   = sum_{e->v} T[col_e]            (pure gather + segment-sum, no
                                             per-edge arithmetic at all)
    T'[v]  = (dis*dis_sh)[v]*S[v] + (dis*xX)[v]   (one DVE op per group)

Distribution: nodes dst-sharded over 8 cores; edges partitioned by dst core.
Nodes are relabeled into "slots": 32-slot sub-blocks (bins) packed so each
bin's in-edges fit a variable number of 128-edge chunks; 4 bins = one PSUM
group of 128 dst rows. Per iteration each core stages its T' shard (fp8) and
one AllGather rebuilds the full table on every core at the iteration
boundary. The boundary AG runs in a quiet window (the first gather of the
next iteration head-of-line parks the in-order Pool sequencer until the AG
lands) because concurrent SWDGE gather traffic inflates collective latency
~3x (small random reads keep the DMA engines latency-saturated).

The table is fp8 (e4m3); rows are gathered in PAIRS (256B descriptors - the
dma_gather minimum) and a per-chunk [even|odd] pair of 32-col fp8 selection
blocks picks the right row of each gathered pair on the TensorEngine,
accumulating fp32 in PSUM. Descriptor generation on the 4 SWDGE queues
(~2ns/idx aggregate) and the DMA engines' random-read latency are the
co-limiting resources; total gathered indices are minimized by per-bin
variable chunk counts (cross-core aligned by sorted position).
"""

import numpy as np

# problem shape (hardcoded per the task contract)
N = 50000
E = 800000
IN_C = 128
HID = 128
OUT_C = 40
POWER1 = 8

NCORES = 8
SUB_NODES = 32          # destination slots per sub-block (= matmul M)
CHUNK = 128             # edges per chunk (= matmul K)
GROUP_SUBS = 4          # sub-blocks per psum group ([128,128] psum tile)
CAP_EDGES = 512         # max in-edges per bin (4 chunks)
BATCH_CHUNKS = 8        # max chunks per gather call (1024 idx)
NQUEUES = 4             # SWDGE queues used for the gathers


# ----------------------------------------------------------------------------
# Host-side preprocessing
# ----------------------------------------------------------------------------

def _pack_core(deg, cap, sub_nodes):
    """1D best-fit-decreasing packing of nodes into bins (<=sub_nodes nodes,
    <=cap edges)."""
    order = np.argsort(-deg, kind="stable")
    bins = []        # (node_list, sum_edges)
    for v in order:
        a = int(deg[v])
        best, best_slack = -1, None
        for i, (nodes, sa) in enumerate(bins):
            if len(nodes) < sub_nodes and sa + a <= cap:
                slack = cap - sa - a
                if best_slack is None or slack < best_slack:
                    best, best_slack = i, slack
        if best < 0:
            bins.append(([v], a))
        else:
            nodes, sa = bins[best]
            nodes.append(v)
            bins[best] = (nodes, sa + a)
    return bins


def _preprocess(inputs, n=N, ncores=NCORES):
    import ml_dtypes
    x = np.asarray(inputs["x"], dtype=np.float32)
    edge_index = np.asarray(inputs["edge_index"])
    W_linX = np.asarray(inputs["W_linX"], dtype=np.float32)
    b_linX = np.asarray(inputs["b_linX"], dtype=np.float32)
    policy = np.asarray(inputs["policy"], dtype=np.float64)
    W_pred = np.asarray(inputs["W_pred"], dtype=np.float32)
    b_pred = np.asarray(inputs["b_pred"], dtype=np.float32)

    npc = n // ncores
    row = edge_index[0].astype(np.int64)
    col = edge_index[1].astype(np.int64)
    deg = np.bincount(col, minlength=n).astype(np.float64)
    dis = np.where(deg > 0, 1.0 / np.sqrt(np.maximum(deg, 1.0)), 0.0)
    shift = int(row.min())
    dst = row - shift                      # aggregation destination
    # per-dst factor is dis at the *unshifted* row id
    dis_sh = np.zeros(n, dtype=np.float64)
    hi = n - shift
    dis_sh[:hi] = dis[shift:]

    e = np.exp(policy[:2] - policy[:2].max())
    pp = e / e.sum()
    pp0, pp1 = float(pp[0]), float(pp[1])
    b_comb = pp0 + pp1                      # == 1.0, but don't rely on it

    # ---- per-core 1D bin packing of dst nodes
    cores = []
    for c in range(ncores):
        m = (dst >= c * npc) & (dst < (c + 1) * npc)
        e_dst = dst[m] - c * npc
        e_src = col[m]
        degc = np.bincount(e_dst, minlength=npc)
        bins = _pack_core(degc, CAP_EDGES, SUB_NODES)
        # canonical order: descending chunk count, so per-position maxima
        # across cores stay tight
        bins.sort(key=lambda b: -b[1])
        cores.append((e_dst, e_src, bins))

    max_bins = max(len(b) for _, _, b in cores)
    n_sub = -(-max_bins // GROUP_SUBS) * GROUP_SUBS
    n_grp = n_sub // GROUP_SUBS
    slots = n_sub * SUB_NODES               # table rows per core
    rows_tot = slots * ncores               # rows of the shared table
    assert rows_tot // 2 <= 32767, f"pair count {rows_tot//2} exceeds int16"

    # common per-position chunk counts (>=1 so every psum range is written)
    cnt = np.ones(n_sub, dtype=np.int64)
    for _, _, bins in cores:
        for i, (_, se) in enumerate(bins):
            cnt[i] = max(cnt[i], -(-se // CHUNK))
    off = np.zeros(n_sub + 1, dtype=np.int64)
    np.cumsum(cnt, out=off[1:])
    TC = int(off[n_sub])                    # total chunks per iteration

    # batches: uniform chunk ranges (groups may span several gather calls)
    batches = []                            # (ch0, nch)
    ch = 0
    while ch < TC:
        nch = min(BATCH_CHUNKS, TC - ch)
        batches.append((ch, nch))
        ch += nch
    max_nch = max(b[1] for b in batches)

    # slot assignment + per-core tensors
    slot_of_node = np.full(n, -1, dtype=np.int64)   # local slot within core
    for c, (e_dst, e_src, bins) in enumerate(cores):
        for bi, (nodes, _) in enumerate(bins):
            for k, v in enumerate(nodes):
                slot_of_node[c * npc + v] = bi * SUB_NODES + k
    core_of = np.arange(n) // npc
    # table rows split in two parts at a group boundary so the front part's
    # AllGather can launch mid-iteration: [c0 F | .. | c7 F | c0 B | .. c7 B]
    # front ~63%, back ~37%: both AllGathers run in the boundary quiet
    # window; measured faster than a single merged AG or mid-iteration
    # launches under single_packet coalescing
    n_grp_f = int(round(n_grp * 0.63))
    slots_f = n_grp_f * GROUP_SUBS * SUB_NODES
    RF = ncores * slots_f
    s_loc = slot_of_node
    grow = np.where(s_loc < slots_f,
                    core_of * slots_f + s_loc,
                    RF + core_of * (slots - slots_f) + (s_loc - slots_f))

    per_core = []
    for c, (e_dst, e_src, bins) in enumerate(cores):
        dloc = slot_of_node[e_dst + c * npc]
        dst_bin = dloc // SUB_NODES
        dst_k = dloc % SUB_NODES
        # pair-gather: table rows gathered in PAIRS (256B fp8 descriptors);
        # per chunk an [even | odd] pair of 32-col blocks picks the row.
        S = np.zeros((TC, CHUNK, 2, SUB_NODES), dtype=np.float32)
        idx = np.zeros(TC * CHUNK, dtype=np.int64)  # pad -> pair 0 (S row 0)
        for bi in range(n_sub):
            sel = dst_bin == bi
            srcs = e_src[sel]
            dks = dst_k[sel]
            kk = len(srcs)
            assert kk <= cnt[bi] * CHUNK, (c, bi, kk)
            cbase = off[bi]
            gr = grow[srcs]
            for j in range(kk):
                ch = cbase + j // CHUNK
                S[ch, j % CHUNK, gr[j] % 2, dks[j]] = 1.0
            idx[cbase * CHUNK: cbase * CHUNK + kk] = gr // 2
        assert idx.min() >= 0 and idx.max() < rows_tot // 2

        def wrap_idx(ix):
            # index i consumed from [i % 16, i // 16]; replicate to 128 parts
            w = ix.reshape(-1, 16).T.astype(np.int16)      # [16, n/16]
            return np.tile(w, (8, 1))                       # [128, n/16]

        # slot-layout host arrays (x pre-transposed: [IN_C, slots])
        x_slot = np.zeros((slots, IN_C), dtype=np.float32)
        dis_slot = np.zeros(slots, dtype=np.float64)
        dsh_slot = np.zeros(slots, dtype=np.float64)
        nodes_c = np.arange(c * npc, (c + 1) * npc)
        sl = slot_of_node[nodes_c]
        x_slot[sl] = x[nodes_c]
        dis_slot[sl] = dis[nodes_c]
        dsh_slot[sl] = dis_sh[nodes_c]

        grp = lambda v: v.reshape(n_grp, 128).T.astype(np.float32)
        per_core.append({
            "x_slot": np.ascontiguousarray(x_slot.T),
            "S": np.ascontiguousarray(
                S.transpose(1, 0, 2, 3).reshape(CHUNK, -1)
                .astype(ml_dtypes.float8_e4m3)),             # [128, TC*64]
            "idx": wrap_idx(idx),
            "disg": grp(dis_slot),
            "dis2g": grp(dis_slot * dsh_slot),
            "ag": grp((pp1 / b_comb) * dsh_slot),
            "W_linX": W_linX,
            "bX": np.tile(b_linX[None, :], (128, 1)).astype(np.float32),
            "W_pred": (b_comb * W_pred).astype(np.float32),
            "bP": np.tile(b_pred[None, :], (128, 1)).astype(np.float32),
            "ident": np.eye(128, dtype=np.float32),
        })

    meta = dict(n=n, ncores=ncores, npc=npc, n_sub=n_sub, n_grp=n_grp,
                slots=slots, rows_tot=rows_tot, TC=TC,
                cnt=cnt, off=off, batches=batches, max_nch=max_nch,
                n_grp_f=n_grp_f, slots_f=slots_f, RF=RF,
                slot_of_node=slot_of_node)
    return meta, per_core


# ----------------------------------------------------------------------------
# Bass program
# ----------------------------------------------------------------------------

def _build_program(meta, iters=POWER1):
    import concourse.bacc as bacc
    import concourse.mybir as mybir
    from concourse import tile

    f32, f16, i16 = mybir.dt.float32, mybir.dt.float16, mybir.dt.int16
    f8 = mybir.dt.float8e4
    ADD, MULT = mybir.AluOpType.add, mybir.AluOpType.mult

    ncores = meta["ncores"]
    n_sub, n_grp = meta["n_sub"], meta["n_grp"]
    slots, rows_tot = meta["slots"], meta["rows_tot"]
    TC = meta["TC"]
    cnt, off = meta["cnt"], meta["off"]
    batches, max_nch = meta["batches"], meta["max_nch"]
    n_grp_f, slots_f, RF = meta["n_grp_f"], meta["slots_f"], meta["RF"]
    # first batch index by which every front-part group has been consumed
    bF = next(b for b, (ch0, nch) in enumerate(batches)
              if off[n_grp_f * GROUP_SUBS] <= ch0 + nch)

    nc = bacc.Bacc("TRN2", target_bir_lowering=False, debug=False,
                   enable_asserts=False, num_devices=ncores,
                   num_swdge_queues=NQUEUES,
                   dynamic_dma_scratch_size=32768)

    x_slot_h = nc.dram_tensor("x_slot", [IN_C, slots], f32, kind="ExternalInput")
    S_h = nc.dram_tensor("S", [CHUNK, TC * 2 * SUB_NODES], f8,
                         kind="ExternalInput")
    idx_h = nc.dram_tensor("idx", [128, TC * CHUNK // 16], i16,
                           kind="ExternalInput")
    disg_h = nc.dram_tensor("disg", [128, n_grp], f32, kind="ExternalInput")
    dis2g_h = nc.dram_tensor("dis2g", [128, n_grp], f32, kind="ExternalInput")
    ag_h = nc.dram_tensor("ag", [128, n_grp], f32, kind="ExternalInput")
    W_h = nc.dram_tensor("W_linX", [IN_C, HID], f32, kind="ExternalInput")
    bX_h = nc.dram_tensor("bX", [128, HID], f32, kind="ExternalInput")
    Wp_h = nc.dram_tensor("W_pred", [HID, OUT_C], f32, kind="ExternalInput")
    bP_h = nc.dram_tensor("bP", [128, OUT_C], f32, kind="ExternalInput")
    id_h = nc.dram_tensor("ident", [128, 128], f32, kind="ExternalInput")

    # ping-pong shared table (fp8, viewed as row PAIRS for 256B-elem gathers)
    tab = [nc.dram_tensor(f"tab{p}", [rows_tot // 2, 2 * HID], f8,
                          addr_space="Shared") for p in range(2)]
    shardF = [nc.dram_tensor(f"shardF{p}", [slots_f, HID], f8)
              for p in range(2)]
    shardB = ([nc.dram_tensor(f"shardB{p}", [slots - slots_f, HID], f8)
               for p in range(2)] if slots > slots_f else None)
    out_h = nc.dram_tensor("out", [slots, OUT_C], f32, kind="ExternalOutput")

    rg = [list(range(ncores))]

    def allgather(src, dst_ap):
        nc.gpsimd.collective_compute(
            "AllGather", mybir.AluOpType.bypass, replica_groups=rg,
            ins=[src.ap().opt()], outs=[dst_ap.opt()])

    with tile.TileContext(nc, num_cores=ncores) as tc:
        import contextlib
        with contextlib.ExitStack() as ctx:
            cpool = ctx.enter_context(tc.tile_pool(name="const", bufs=1))
            wpool = ctx.enter_context(tc.tile_pool(name="work", bufs=2))
            xpool = ctx.enter_context(tc.tile_pool(name="xload", bufs=1))
            gpool = ctx.enter_context(tc.tile_pool(name="gath", bufs=16))
            spool = ctx.enter_context(tc.tile_pool(name="stage", bufs=6))
            ppool = ctx.enter_context(
                tc.tile_pool(name="psum", bufs=6, space="PSUM"))
            tpool = ctx.enter_context(
                tc.tile_pool(name="psum2", bufs=1, space="PSUM"))

            # persistent SBUF
            S_sb = cpool.tile([CHUNK, TC * 2 * SUB_NODES], f8)
            nc.sync.dma_start(S_sb[:, :], S_h[:, :])
            idx_sb = cpool.tile([128, TC * CHUNK // 16], i16)
            nc.sync.dma_start(idx_sb[:, :], idx_h[:, :])
            disg = cpool.tile([128, n_grp], f32)
            nc.sync.dma_start(disg[:, :], disg_h[:, :])
            dis2g = cpool.tile([128, n_grp], f32)
            nc.sync.dma_start(dis2g[:, :], dis2g_h[:, :])
            ag = cpool.tile([128, n_grp], f32)
            nc.sync.dma_start(ag[:, :], ag_h[:, :])
            W_sb = cpool.tile([IN_C, HID], f32)
            nc.sync.dma_start(W_sb[:, :], W_h[:, :])
            bX_sb = cpool.tile([128, HID], f32)
            nc.sync.dma_start(bX_sb[:, :], bX_h[:, :])
            Wp_sb = cpool.tile([HID, OUT_C], f32)
            nc.sync.dma_start(Wp_sb[:, :], Wp_h[:, :])
            bP_sb = cpool.tile([128, OUT_C], f32)
            nc.sync.dma_start(bP_sb[:, :], bP_h[:, :])
            ident = cpool.tile([128, 128], f32)
            nc.sync.dma_start(ident[:, :], id_h[:, :])
            xX_sb = cpool.tile([128, n_grp * HID], f32)     # computed below
            dxX_sb = cpool.tile([128, n_grp * HID], f32)    # disg * xX

            # ---- prologue: xX = x @ W + b; T0 = dis * xX -> shard
            xT_sb = xpool.tile([128, n_grp * 128], f32, tag="xT")
            nc.sync.dma_start(xT_sb[:, :], x_slot_h[:, :])
            for g in range(n_grp):
                rows = slice(g * 128, (g + 1) * 128)
                gc = slice(g * HID, (g + 1) * HID)
                mm_ps = ppool.tile([128, HID], f32, tag="ps")
                nc.tensor.matmul(mm_ps[:, :],
                                 xT_sb[:, g * 128:(g + 1) * 128],
                                 W_sb[:, :], start=True, stop=True)
                nc.vector.tensor_tensor(xX_sb[:, gc], mm_ps[:, :],
                                        bX_sb[:, :], op=ADD)
                nc.vector.tensor_scalar_mul(dxX_sb[:, gc], xX_sb[:, gc],
                                            disg[:, g:g + 1])
                stage = spool.tile([128, HID], f8, tag="stage")
                nc.vector.tensor_copy(stage[:, :], dxX_sb[:, gc])
                if g < n_grp_f:
                    nc.sync.dma_start(shardF[0][rows, :], stage[:, :])
                else:
                    rb = slice((g - n_grp_f) * 128, (g - n_grp_f + 1) * 128)
                    nc.sync.dma_start(shardB[0][rb, :], stage[:, :])
                if g == n_grp_f - 1:
                    allgather(shardF[0], tab[0][0:RF // 2, :])

            # ---- 8 SpMM iterations.
            # The FRONT part's AllGather for iteration t+1 launches
            # mid-iteration t (contended with gather traffic but hidden);
            # only the BACK part's AllGather sits in the boundary quiet
            # window (G#0 of the next iteration head-of-line parks the
            # in-order Pool sequencer until it lands - concurrent gathers
            # would inflate the collective ~3x).
            for t in range(iters):
                p = t % 2
                last = t == iters - 1
                if shardB is not None:
                    allgather(shardB[p], tab[p][RF // 2:rows_tot // 2, :])
                tiles = {}
                gdone = 0
                for b, (ch0, nch) in enumerate(batches):
                    nidx = nch * CHUNK
                    mg = gpool.tile([128, max_nch * CHUNK * 2], f8, tag="mg")
                    tiles[b] = mg
                    nc.gpsimd.dma_gather(
                        mg[:, :nidx * 2].rearrange("p (c e) -> p c e",
                                                   e=2 * HID),
                        tab[p][0:rows_tot // 2, :],
                        idx_sb[:, ch0 * (CHUNK // 16):
                               (ch0 + nch) * (CHUNK // 16)],
                        num_idxs=nidx, num_idxs_reg=nidx,
                        elem_size=2 * HID, single_packet=True,
                        queue_num=b % NQUEUES)
                    end = ch0 + nch
                    while (gdone < n_grp
                           and off[(gdone + 1) * GROUP_SUBS] <= end):
                        g = gdone
                        gdone += 1
                        gc = slice(g * HID, (g + 1) * HID)
                        ps = ppool.tile([128, HID], f32, tag="ps")
                        for j in range(GROUP_SUBS):
                            sb = g * GROUP_SUBS + j
                            prange = slice(32 * j, 32 * j + 32)
                            tpos = (0, 32 * j)
                            nch_b = int(cnt[sb])
                            for k in range(nch_b):
                                cg = int(off[sb]) + k          # global chunk
                                bb = cg // BATCH_CHUNKS        # owning batch
                                q = cg - bb * BATCH_CHUNKS
                                mgb = tiles[bb]
                                for par in range(2):
                                    nc.tensor.matmul(
                                        ps[prange, :],
                                        S_sb[:, cg * 64 + par * 32:
                                             cg * 64 + par * 32 + 32],
                                        mgb[:, q * 256 + par * HID:
                                            q * 256 + (par + 1) * HID],
                                        start=(k == 0 and par == 0),
                                        stop=(k == nch_b - 1 and par == 1),
                                        tile_position=tpos)
                        if not last:
                            stage = spool.tile([128, HID], f8, tag="stage")
                            nc.vector.scalar_tensor_tensor(
                                stage[:, :], ps[:, :], dis2g[:, g:g + 1],
                                dxX_sb[:, gc], op0=MULT, op1=ADD)
                            pn = (t + 1) % 2
                            if g < n_grp_f:
                                rows = slice(g * 128, (g + 1) * 128)
                                nc.sync.dma_start(shardF[pn][rows, :],
                                                  stage[:, :])
                            else:
                                rb = slice((g - n_grp_f) * 128,
                                           (g - n_grp_f + 1) * 128)
                                nc.sync.dma_start(shardB[pn][rb, :],
                                                  stage[:, :])
                        else:
                            u_t = wpool.tile([128, HID], f32, tag="t1")
                            nc.vector.scalar_tensor_tensor(
                                u_t[:, :], ps[:, :], ag[:, g:g + 1],
                                xX_sb[:, gc], op0=MULT, op1=ADD)
                            nc.vector.tensor_scalar_max(u_t[:, :], u_t[:, :],
                                                        0.0)
                            tp_ps = tpool.tile([128, 128], f32, tag="tp")
                            nc.tensor.transpose(tp_ps[:, :], u_t[:, :],
                                                ident[:, :])
                            uT_sb = wpool.tile([128, 128], f32, tag="xT")
                            nc.vector.tensor_copy(uT_sb[:, :], tp_ps[:, :])
                            o_ps = tpool.tile([128, OUT_C], f32, tag="mm2")
                            nc.tensor.matmul(o_ps[:, :], uT_sb[:, :],
                                             Wp_sb[:, :], start=True, stop=True)
                            o_sb = spool.tile([128, OUT_C], f32, tag="osb")
                            nc.vector.tensor_tensor(o_sb[:, :], o_ps[:, :],
                                                    bP_sb[:, :], op=ADD)
                            rows = slice(g * 128, (g + 1) * 128)
                            nc.sync.dma_start(out_h[rows, :], o_sb[:, :])
                    if not last and b == bF:
                        allgather(shardF[(t + 1) % 2],
                                  tab[(t + 1) % 2][0:RF // 2, :])

    nc.compile()
    return nc


# ----------------------------------------------------------------------------
# Runner
# ----------------------------------------------------------------------------

def _run(inputs, n=N, ncores=NCORES, trace=False, use_sim=False, iters=POWER1):
    meta, per_core = _preprocess(inputs, n=n, ncores=ncores)
    nc = _build_program(meta, iters=iters)
    in_maps = [dict(pc) for pc in per_core]

    if use_sim:
        from concourse.bass_interp import MultiCoreSim
        sim = MultiCoreSim(nc, num_cores=ncores)
        for c in range(ncores):
            for k, v in in_maps[c].items():
                sim.cores[c].tensor(k)[:] = v
        sim.simulate(check_with_hw=False)
        results = [{"out": np.array(sim.cores[c].tensor("out"))}
                   for c in range(ncores)]
        bres = None
    else:
        from concourse.bass_utils import run_bass_kernel_spmd
        bres = run_bass_kernel_spmd(nc, in_maps, core_ids=list(range(ncores)),
                                    trace=trace)
        results = bres.results

    # unshard: slots -> nodes
    npc = meta["npc"]
    son = meta["slot_of_node"]
    out = np.zeros((n, OUT_C), dtype=np.float32)
    for c in range(ncores):
        nodes = np.arange(c * npc, (c + 1) * npc)
        out[nodes] = results[c]["out"][son[nodes]]
    return out, bres


def kernel(**inputs) -> np.ndarray:
    # Run twice and compare: guards against rare transient device faults
    # (observed once after an unrecoverable-NRT event on a shared terminal).
    out1, _ = _run(inputs)
    out2, _ = _run(inputs)
    if np.allclose(out1, out2, rtol=0, atol=1e-4):
        return out1
    out3, _ = _run(inputs)
    if np.allclose(out1, out3, rtol=0, atol=1e-4):
        return out1
    return out2 if np.allclose(out2, out3, rtol=0, atol=1e-4) else out3
